# revision 55
# baseline (speedup 1.0000x reference)
"""Trainium2 Bass kernel for nn_Net_76330158785143 (dense_cnn).

Pipeline per sample: per-sample 11x11 autocorrelation of channel 2 ->
conv5x5(1->32) relu -> maxpool2 -> conv5x5(32->64) relu -> maxpool2 ->
conv3x3(64->10) relu -> GAP -> log_softmax.

Sharding: pure data parallel, batch 8192 -> 1024 per core across 8 cores.

PE work is minimized by K/M-packing (matmul cost ~ out free size only):
 - corr: fp8(e4m3) DoubleRow diag matmuls, two taps per pass (pairs along
   x, plus pairs along y for the leftover column), each clipped to the
   union sub-rectangle its zero-padded shifts can reach. The diag
   matrices and the padded fp8 image are precomputed on the host and
   streamed in. Drained to an fp8 corr map on ACT (the next tile's first
   corr matmul waits on this; the DVE queue would bury it behind pools).
 - conv1: im2col via 40 shift-replicated tap partitions staged in fp8 at
   partitions 64..103 dy-MAJOR (partition = 64 + 5*dy + dx): the 8
   per-dy SBUF->SBUF gathers land on 8 distinct DMA ports, and the 4 dx
   shifts are partition-stride-5 contiguous copies. Staged 2 subs ahead
   (3 bufs); gathers split sync/gpsimd rings, shifts+replicas on the
   gpsimd (SWDGE) ring so the ACT/sync HWDGE rings stay short. K=40,
   M=128=(4 out rows x 32 ch), bf16 weights x fp8 moving. The matmul
   PSUM out-AP de-interleaves x-parity (free for the PE), so the
   relu+bias drains on ACT are fully contiguous and the maxpool runs as
   packed-2x DVE ops at equal partition bases.
 - conv2: 4 quarter-shifted input replicas {0,1 col; 0,1 row} give
   K=128=(2dy x 2dx x 32 ci), M=128=(2 out rows x 64 ch): 9 accumulating
   matmuls of N=512 per cc, bf16. PSUM out-AP de-interleaves x-parity;
   relu+bias on ACT straight from PSUM is contiguous; pool y then pool x
   run on DVE, the x-fold written twice straight into the l3 staging
   layout (rep1 shifted 1 pixel) so no l3 DMAs are needed.
 - conv3: K=128=(2 input-pixel reps, 64 ci), M=128=(4 out pix x co),
   8 matmuls of N=32; GAP 1/4 scale folded into the weights; drains on
   DVE to keep the ACT queue short for the PSUM-gating drains.
Scheduling: a flat rolling pipeline emits the corr matmuls of tile k
interleaved into the conv phases of tile k-1 (B-phases lag A-phases by
two sub-slots, crossing tile boundaries), so the PE fills every
ACT/pool/DMA-gated gap; img8/dgall prefetch one tile ahead of use.
"""

import sys

sys.path.insert(0, "/opt/trn_rl_repo")

import numpy as np

import concourse.bacc as bacc
import concourse.mybir as mybir
from concourse.ap import AP
from concourse.tile import TileContext
from concourse.bass_utils import run_bass_kernel_spmd

F32 = mybir.dt.float32
BF16 = mybir.dt.bfloat16
FP8 = mybir.dt.float8e4
DR = mybir.MatmulPerfMode.DoubleRow
ALU = mybir.AluOpType
ACTF = mybir.ActivationFunctionType
AXIS = mybir.AxisListType

N_CORES = 8
B_FULL = 8192
B_CORE = B_FULL // N_CORES

S40P = 18816  # s40 per-partition pitch: 32 samples x 588-elem windows
S40B = 64  # s40/w1e base partition (odd DMA ports, frees even for the rest)

# corr work units: (u, v, paired). Pairs (v, v+1) share one DoubleRow
# matmul; the unit containing the center tap (u=5, v=4/5 -> full union
# rect) is emitted first so start=True zeroes the whole PSUM bank.
_UNITS = []
for _u in range(11):
    for _v in (0, 2, 4, 6, 8):
        _UNITS.append((_u, _v, "v"))
for _u in (0, 2, 4, 6, 8):
    _UNITS.append((_u, 10, "u"))
_UNITS.append((10, 10, None))
_UNITS.remove((5, 4, "v"))
_UNITS.insert(0, (5, 4, "v"))


def _build(nc, b_core):
    """Emit the full per-core program for b_core samples (multiple of 128)."""
    n_bt = b_core // 128  # 128-sample tiles

    img8_d = nc.dram_tensor("img8p", [b_core, 1444], FP8, kind="ExternalInput")
    dgall_d = nc.dram_tensor("dgallp", [b_core, 15488], FP8, kind="ExternalInput")
    ident10p_d = nc.dram_tensor("ident10p", [16, 16], F32, kind="ExternalInput")
    w1e_d = nc.dram_tensor("w1e", [40, 128], BF16, kind="ExternalInput")
    w2qp_d = nc.dram_tensor("w2qp", [128, 1152], BF16, kind="ExternalInput")
    w3e_d = nc.dram_tensor("w3e", [128, 1024], BF16, kind="ExternalInput")
    b1p_d = nc.dram_tensor("b1p", [128, 1], F32, kind="ExternalInput")
    b2p_d = nc.dram_tensor("b2p", [64, 1], F32, kind="ExternalInput")
    b3q_d = nc.dram_tensor("b3q", [16, 1], F32, kind="ExternalInput")
    out_d = nc.dram_tensor("out", [b_core, 10], F32, kind="ExternalOutput")

    with TileContext(nc) as tc:
        cpool_cm = tc.tile_pool(name="const", bufs=1)
        cpool = cpool_cm.__enter__()

        def _load_const(name, dram, shape, dtype):
            t = cpool.tile(shape, dtype, name=name + "_sb")
            f = int(np.prod(shape[1:]))
            nc.sync.dma_start(
                out=AP(t.tensor, 0, [[f, shape[0]], [1, f]]),
                in_=AP(dram, 0, [[f, shape[0]], [1, f]]),
            )
            return t

        def _load_consts():
            # w1e lives at partitions S40B..S40B+39 to match the s40 K-window
            w1e_t = cpool.tile([128, 128], BF16, name="w1e_sb")
            nc.sync.dma_start(
                out=AP(w1e_t.tensor, S40B * 128, [[128, 40], [1, 128]]),
                in_=AP(w1e_d, 0, [[128, 40], [1, 128]]),
            )
            return dict(
                ident10=_load_const("ident10", ident10p_d, [16, 16], F32),
                w1e_sb=w1e_t,
                w2q_sb=_load_const("w2qp", w2qp_d, [128, 1152], BF16),
                w3e_sb=_load_const("w3e", w3e_d, [128, 1024], BF16),
                b1_sb=_load_const("b1p", b1p_d, [128, 1], F32),
                b2_sb=_load_const("b2p", b2p_d, [64, 1], F32),
                b3q_sb=_load_const("b3q", b3q_d, [16, 1], F32),
            )

        C = {}

        from contextlib import ExitStack

        with ExitStack() as stack:
            def _pool(name, bufs, space=None):
                kw = {"space": space} if space else {}
                return stack.enter_context(tc.tile_pool(name=name, bufs=bufs, **kw))

            P = dict(
                img8pool=_pool("img8", 2),
                dgallpool=_pool("dgall", 2),
                corrpool=_pool("corr", 2),
                s40pool=_pool("s40", 3),
                a1spool=_pool("a1s", 1),
                a1s2pool=_pool("a1s2", 1),
                pxspool=_pool("pxs", 1),
                dupApool=_pool("dupA", 3),
                m1pool=_pool("m1", 2),
                l3pool=_pool("l3", 2),
                smpool=_pool("sm", 2),
                lgbpool=_pool("lgb", 2),
                smfpool=_pool("smf", 1),
                pcorr=_pool("pcorr", 1, "PSUM"),
                pc1=_pool("pc1", 3, "PSUM"),
                pc2=_pool("pc2", 2, "PSUM"),
                pc3=_pool("pc3", 1, "PSUM"),
            )

            # Flat rolling pipeline. Per steady iteration k:
            #   P1: A0(k-1) B2(k-2) A1(k-1) B3(k-2) A2(k-1) B0(k-1),
            #       corr units of bt k filling the conv pipeline gaps
            #   P2: drain corr units, finish(k) -> corr(k), create
            #       state(k) (which prefetches s40(k,0) ahead of the
            #       replica DMAs emitted later in A3)
            #   P3: setup(k+1) so its units can fill P4
            #   P4: A3(k-1) B1(k-1)
            states = {}

            def mkfill(ce):
                it = iter(range(61))

                def fill(n, _it=it, _ce=ce):
                    got = []
                    for _ in range(n):
                        i = next(_it, None)
                        if i is None:
                            break
                        got.append(i)
                    if got:
                        _ce.units(got[0], got[-1] + 1)

                return fill

            def run(st, kind, s, fill):
                if st is None:
                    return
                if kind == "a":
                    st.phase_a(s, fill)
                else:
                    st.phase_b(s, fill)
                    if s == 3:
                        st.lgall = lgall
                        st.tail()
                        del states[st.bt]

            lgall = P["smfpool"].tile([128, 128], F32, tag="lgall")
            ce = _CorrEmitter(nc, 0, img8_d, dgall_d, P, C)
            ce.setup()
            # consts load after the first tile's img8/dgall so the first
            # corr matmuls aren't stalled behind them on the DMA queue
            C.update(_load_consts())
            fill = mkfill(ce)
            for k in range(n_bt):
                sa = states.get(k - 1)
                sb2 = states.get(k - 2)
                for st, kind, s in (
                    (sb2, "b", 2), (sa, "a", 0),
                    (sb2, "b", 3), (sa, "a", 1),
                    (sa, "b", 0), (sa, "a", 2),
                ):
                    run(st, kind, s, fill)
                # queue the next tile's img8/dgall loads ahead of the corr
                # drain block so they overlap ~30us of PE work before the
                # first unit of tile k+1 needs them
                ce_next = None
                if k + 1 < n_bt:
                    ce_next = _CorrEmitter(nc, k + 1, img8_d, dgall_d, P, C)
                    ce_next.setup()
                fill(61)
                corr = ce.finish()
                if ce_next is not None:
                    ce = ce_next
                    fill = mkfill(ce)
                else:
                    fill = None
                states[k] = _ConvState(nc, k, corr, out_d, P, C)
                run(sa, "b", 1, fill)
                run(sa, "a", 3, fill)
            # drain: conv phases of the last two tiles, no corr fill left
            for k in (n_bt, n_bt + 1):
                sa = states.get(k - 1)
                sb2 = states.get(k - 2)
                for st, kind, s in (
                    (sb2, "b", 2), (sa, "a", 0),
                    (sb2, "b", 3), (sa, "a", 1),
                    (sa, "b", 0), (sa, "a", 2),
                    (sa, "b", 1), (sa, "a", 3),
                ):
                    run(st, kind, s, None)
            _softmax_final(nc, n_bt, lgall, out_d, P)
        cpool_cm.__exit__(None, None, None)
    return nc


class _CorrEmitter:
    """Per-sample 11x11 correlation of 128 samples via fp8 DoubleRow."""

    def __init__(self, nc, bt, img8_d, dgall_d, P, C):
        self.nc = nc
        self.bt = bt
        self.img8_d = img8_d
        self.dgall_d = dgall_d
        self.P = P
        self.C = C

    def setup(self):
        nc, P = self.nc, self.P
        img8 = P["img8pool"].tile([128, 1444], FP8)
        nc.sync.dma_start(
            out=AP(img8.tensor, 0, [[1444, 128], [1, 1444]]),
            in_=AP(self.img8_d, self.bt * 128 * 1444, [[1444, 128], [1, 1444]]),
        )
        dgall = P["dgallpool"].tile([128, 15488], FP8)
        nc.sync.dma_start(
            out=AP(dgall.tensor, 0, [[15488, 128], [1, 15488]]),
            in_=AP(self.dgall_d, self.bt * 128 * 15488, [[15488, 128], [1, 15488]]),
        )
        self.dgall = dgall
        self.img8 = img8
        self.ps_a = P["pcorr"].tile([128, 392], F32, tag="corr_a")
        self.ps_b = P["pcorr"].tile([128, 392], F32, tag="corr_b")

    def units(self, lo, hi):
        nc = self.nc
        img8 = self.img8
        for i in range(lo, hi):
            u, v, paired = _UNITS[i]
            # valid output rows r: img_pad rows r+u in [5, 32]; "v" pairs
            # (v, v+1) in one DoubleRow pass, "u" pairs (u, u+1)
            if paired == "u":
                r0 = min(max(0, 5 - u), max(0, 5 - u - 1))
                r1 = max(min(27, 32 - u), min(27, 32 - u - 1))
            else:
                r0, r1 = max(0, 5 - u), min(27, 32 - u)
            if paired == "v":
                c0 = min(max(0, 5 - v), max(0, 5 - v - 1))
                c1 = max(min(27, 32 - v), min(27, 32 - v - 1))
            else:
                c0, c1 = max(0, 5 - v), min(27, 32 - v)
            ncol = c1 - c0 + 1
            t = u * 11 + v
            if paired == "v":
                lhs = AP(self.dgall.tensor, t * 128, [[15488, 128], [128, 2], [1, 128]])
            elif paired == "u":
                lhs = AP(self.dgall.tensor, t * 128, [[15488, 128], [11 * 128, 2], [1, 128]])
            else:
                lhs = AP(self.dgall.tensor, t * 128, [[15488, 128], [1, 128]])
            kst = {"v": 1, "u": 38}.get(paired)
            for bank, (b0, b1, ps) in enumerate(
                ((r0, min(r1, 13), self.ps_a), (max(r0, 14), r1, self.ps_b))
            ):
                if b0 > b1:
                    continue
                nr = b1 - b0 + 1
                out = AP(
                    ps.tensor,
                    (b0 - 14 * bank) * 28 + c0,
                    [[392, 128], [28, nr], [1, ncol]],
                )
                if paired:
                    rhs = AP(
                        img8.tensor,
                        (u + b0) * 38 + v + c0,
                        [[1444, 128], [kst, 2], [38, nr], [1, ncol]],
                    )
                    nc.tensor.matmul(
                        out, lhs, rhs,
                        start=(i == 0), stop=(i == 60),
                        perf_mode=DR, skip_group_check=True,
                    )
                else:
                    rhs = AP(
                        img8.tensor,
                        (u + b0) * 38 + v + c0,
                        [[1444, 128], [38, nr], [1, ncol]],
                    )
                    nc.tensor.matmul(
                        out, lhs, rhs,
                        start=False, stop=(i == 60),
                        skip_group_check=True,
                    )

    def finish(self):
        nc, P = self.nc, self.P
        # fp8 corr map: feeds conv1's moving operand (weights stay bf16);
        # halves the im2col staging bytes. Drain on ACT: the next tile's
        # first corr matmul waits on these (pcorr WAR), and the DVE queue
        # buries them behind pool bursts while ACT drains promptly.
        corr = P["corrpool"].tile([128, 784], FP8)
        nc.scalar.activation(corr[:, 0:392], self.ps_a[:, :], ACTF.Copy)
        nc.scalar.activation(corr[:, 392:784], self.ps_b[:, :], ACTF.Copy)
        return corr


class _ConvState:
    """conv1 -> pool -> conv2 -> pool -> conv3 -> GAP -> log_softmax for one
    128-sample tile, split into per-32-sample A (im2col+conv1+pool+replicas)
    and B (conv2+pool+conv3+GAP) phases."""

    def __init__(self, nc, bt, corr, out_d, P, C):
        self.nc = nc
        self.bt = bt
        self.corr = corr
        self.out_d = out_d
        self.P = P
        self.C = C
        self.logitsb = P["lgbpool"].tile([16, 128], F32)
        self.dups = [None] * 4
        self.s40s = [None] * 4
        self._s40(0)
        self._s40(1)

    def _s40(self, sub):
        """Stage the conv1 im2col window for one 32-sample sub at partitions
        S40B..S40B+39, dy-MAJOR (partition = S40B + 5*dy + dx): one dy-
        gather from the DRAM corr copy (dest partitions stride 5 land on 8
        distinct DMA ports), then 4 partition-stride-5 contiguous dx-shift
        copies (tail elements that wrap into the next sample's window are
        never read). Shifts alternate the two HWDGE rings (sync/scalar)."""
        nc, P = self.nc, self.P
        s40 = P["s40pool"].tile([128, S40P], FP8)
        for dy in range(8):
            eng = nc.sync if dy % 2 == 0 else nc.gpsimd
            eng.dma_start(
                out=AP(s40.tensor, (S40B + 5 * dy) * S40P, [[S40P, 1], [1, S40P]]),
                in_=AP(
                    self.corr.tensor,
                    sub * 32 * 784 + dy * 28,
                    [[784, 32], [1, 588]],
                ),
            )
        for dx in range(1, 5):
            nc.gpsimd.dma_start(
                out=AP(s40.tensor, (S40B + dx) * S40P, [[5 * S40P, 8], [1, S40P - 4]]),
                in_=AP(s40.tensor, S40B * S40P + dx, [[5 * S40P, 8], [1, S40P - 4]]),
            )
        self.s40s[sub] = s40

    def phase_a(self, sub, fill=None):
        nc, P, C = self.nc, self.P, self.C
        if sub < 2:
            self._s40(sub + 2)  # prefetch 2 subs ahead (s40pool bufs=3)
        s40 = self.s40s[sub]
        # conv1: K=40 (partitions S40B..), M=(rm4, co32); the PSUM out-AP
        # de-interleaves x-parity so drains are contiguous and land in the
        # two half tiles (samples 0-17 / 18-31) at equal partition bases.
        dupA = P["dupApool"].tile([128, 4624], BF16)
        nc.gpsimd.memset(dupA[0:32, 4608:4624], 0.0)
        a1A = P["a1spool"].tile([128, 2592], BF16)
        a1B = P["a1s2pool"].tile([128, 2016], BF16)
        for g in range(11):
            s0 = 3 * g
            ns = 3 if g < 10 else 2
            nf = ns * 144
            ps1 = P["pc1"].tile([128, 432], F32, tag="ps1")
            nc.tensor.matmul(
                AP(ps1.tensor, 0, [[432, 128], [12, ns * 6], [1, 12], [ns * 72, 2]]),
                C["w1e_sb"][S40B : S40B + 40, :],
                AP(
                    s40.tensor,
                    S40B * S40P + s0 * 588,
                    [[S40P, 40], [588, ns], [112, 6], [1, 24]],
                ),
                start=True,
                stop=True,
            )
            # relu+bias, contiguous PSUM read / 216-elem runs out. Mostly on
            # ACT (the PSUM-bank WAR for matmul g+3 waits on drain g); g 1
            # and 3 go to DVE -- emitted before any pool burst enters the
            # DVE queue, so they complete promptly and shorten ACT's serial
            # drain chain that paces the conv1 matmuls.
            half, hoff, hp = (a1A, g * 216, 2592) if g < 6 else (a1B, (g - 6) * 216, 2016)
            if g in (1, 3):
                nc.vector.tensor_scalar(
                    AP(half.tensor, hoff, [[hp, 128], [hp // 2, 2], [1, ns * 72]]),
                    AP(ps1.tensor, 0, [[432, 128], [1, nf]]),
                    C["b1_sb"][:, 0:1], 0.0, ALU.add, ALU.max,
                )
            else:
                nc.scalar.activation(
                    AP(half.tensor, hoff, [[hp, 128], [hp // 2, 2], [1, ns * 72]]),
                    AP(ps1.tensor, 0, [[432, 128], [1, nf]]),
                    ACTF.Relu, bias=C["b1_sb"][:, 0:1],
                )
            if g == 5:
                self._pools(0, 108, a1A, 2592, dupA)
            elif g == 10:
                self._pools(108, 84, a1B, 2016, dupA)
                # replicas for 4-tap K-packing: group g=(gy,gx) holds the
                # pooled map shifted by 12*gy + gx elems
                for g2, sh in ((1, 1), (2, 12), (3, 13)):
                    nc.gpsimd.dma_start(
                        out=AP(dupA.tensor, 32 * g2 * 4624, [[4624, 32], [1, 4596]]),
                        in_=AP(dupA.tensor, sh, [[4624, 32], [1, 4596]]),
                    )
            if fill is not None and (g % 2 == 1 or g == 10):
                fill(3 if g == 10 else 1)
        self.dups[sub] = dupA

    def _pools(self, sq0, nsq, a1h, hp, dupA):
        """maxpool 2x2 over nsq (sample,quad) units starting at fused unit
        sq0. conv1's M layout interleaves row-pairs [rm0, rm2, rm1, rm3],
        so x-pooling runs as two 64-wide ops (per-partition cost only) and
        the y-folds pair the two x-halves of one tile at equal bases
        (parts 0:32 = rm0 vs rm1 -> even rows, 32:64 = rm2 vs rm3 -> odd)."""
        nc, P = self.nc, self.P
        pxall = P["pxspool"].tile([64, 4608], BF16, tag="pxall")
        for grp in range(2):
            nc.vector.tensor_max(
                AP(pxall.tensor, grp * 2304 + sq0 * 12, [[4608, 64], [1, nsq * 12]]),
                AP(a1h.tensor, grp * 64 * hp, [[hp, 64], [1, nsq * 12]]),
                AP(a1h.tensor, grp * 64 * hp + hp // 2, [[hp, 64], [1, nsq * 12]]),
            )
        nc.vector.tensor_max(
            AP(dupA.tensor, sq0 * 24, [[4624, 32], [24, nsq], [1, 12]]),
            AP(pxall.tensor, sq0 * 12, [[4608, 32], [12, nsq], [1, 12]]),
            AP(pxall.tensor, 2304 + sq0 * 12, [[4608, 32], [12, nsq], [1, 12]]),
        )
        nc.vector.tensor_max(
            AP(dupA.tensor, sq0 * 24 + 12, [[4624, 32], [24, nsq], [1, 12]]),
            AP(pxall.tensor, 32 * 4608 + sq0 * 12, [[4608, 32], [12, nsq], [1, 12]]),
            AP(pxall.tensor, 32 * 4608 + 2304 + sq0 * 12, [[4608, 32], [12, nsq], [1, 12]]),
        )

    def phase_b(self, sub, fill=None):
        nc, P, C = self.nc, self.P, self.C
        dupA = self.dups[sub]
        l3 = None
        for cc in range(2):
            ps2 = P["pc2"].tile([128, 512], F32, tag="ps2")
            for q in range(9):
                dy2, dx2 = (q // 3) * 2, (q % 3) * 2
                nc.tensor.matmul(
                    AP(ps2.tensor, 0, [[512, 128], [4, 64], [1, 4], [256, 2]]),
                    C["w2q_sb"][:, q * 128 : q * 128 + 128],
                    AP(dupA.tensor, cc * 2304 + dy2 * 12 + dx2,
                       [[4624, 128], [144, 16], [24, 4], [1, 8]]),
                    start=(q == 0),
                    stop=(q == 8),
                )
                # a corr unit between accumulating matmuls targets other
                # PSUM banks, turning the same-bank drain-wait into work
                if fill is not None and q in (2, 5, 7):
                    fill(1)
            # relu+bias straight from PSUM on ACT (relu commutes with the
            # maxpool); the matmul out-AP already de-interleaved x-parity
            # so both the drain and the pool stages below are contiguous
            rr2 = [
                P["m1pool"].tile([64, 512], BF16, tag=f"rr2{h}", name=f"rr2{h}")
                for h in range(2)
            ]
            for h in range(2):
                nc.scalar.activation(
                    rr2[h][:, :],
                    AP(ps2.tensor, h * 64 * 512, [[512, 64], [1, 512]]),
                    ACTF.Relu, bias=C["b2_sb"][:, 0:1],
                )
            # pool y: rm fold; pool x: parity-half fold written TWICE by
            # DVE straight into the l3 staging layout [128=(2 reps, 64ci),
            # (32s,16pix)] -- rep1 shifted by 1 pixel (no DMA needed; only
            # even l3 columns are ever read by the conv3 matmuls)
            m1 = P["m1pool"].tile([64, 512], BF16, tag="m1")
            nc.vector.tensor_max(m1[:, :], rr2[0][:, :], rr2[1][:, :])
            if cc == 0:
                l3 = P["l3pool"].tile([128, 512], BF16, name="l3")
            nc.vector.tensor_max(
                AP(l3.tensor, cc * 256, [[512, 64], [1, 256]]),
                m1[:, 0:256], m1[:, 256:512],
            )
            nc.vector.tensor_max(
                AP(l3.tensor, 64 * 512 + cc * 256, [[512, 64], [1, 255]]),
                m1[:, 1:256], m1[:, 257:512],
            )
            if fill is not None:
                fill(1)
        # conv3: K=128=(rep2, ci64), M=128=(op4 x 32), 8 matmuls of N=32
        ps3 = P["pc3"].tile([128, 48], F32, tag="ps3", name="ps3")
        for ti2 in range(8):
            nc.tensor.matmul(
                ps3[:, 0:32],
                C["w3e_sb"][:, ti2 * 128 : ti2 * 128 + 128],
                AP(l3.tensor, 2 * ti2, [[512, 128], [16, 32]]),
                start=(ti2 == 0),
                stop=(ti2 == 7),
            )
        # relu(ps3 + b3/4) per output pixel, then GAP = sum of the 4 pixels
        rr = [
            P["smpool"].tile([16, 32], F32, tag=f"rr{op}", name=f"rr{op}")
            for op in range(4)
        ]
        for op in range(4):
            nc.vector.tensor_scalar(
                rr[op][0:10, :], ps3[op * 32 : op * 32 + 10, 0:32],
                C["b3q_sb"][0:10, 0:1], 0.0, ALU.add, ALU.max,
            )
        t1 = P["smpool"].tile([16, 32], F32, tag="t1")
        nc.vector.tensor_add(t1[0:10, :], rr[0][0:10, :], rr[1][0:10, :])
        t2 = P["smpool"].tile([16, 32], F32, tag="t2")
        nc.vector.tensor_add(t2[0:10, :], rr[2][0:10, :], rr[3][0:10, :])
        nc.vector.tensor_add(
            self.logitsb[0:10, sub * 32 : sub * 32 + 32], t1[0:10, :], t2[0:10, :]
        )

    def tail(self):
        nc, P, C = self.nc, self.P, self.C
        psT = P["pc3"].tile([128, 48], F32, tag="ps3", name="psT")
        nc.tensor.transpose(psT[:, 0:10], self.logitsb[0:10, :], C["ident10"][0:10, 0:10])
        nc.vector.tensor_copy(
            out=AP(self.lgall.tensor, self.bt * 16, [[128, 128], [1, 10]]),
            in_=psT[:, 0:10],
        )


def _softmax_final(nc, n_bt, lgall, out_d, P):
    """Batched log_softmax for all tiles: groups each activation function
    into one run so the ACT table loads twice total instead of ~3x per
    tile mid-stream."""
    mxall = P["smfpool"].tile([128, 8], F32, tag="mxall")
    hsall = P["smfpool"].tile([128, 128], F32, tag="hsall")
    exall = P["smfpool"].tile([128, 128], F32, tag="exall")
    smal = P["smfpool"].tile([128, 8], F32, tag="smal")
    lnal = P["smfpool"].tile([128, 8], F32, tag="lnal")
    for bt in range(n_bt):
        nc.vector.reduce_max(
            mxall[:, bt : bt + 1],
            AP(lgall.tensor, bt * 16, [[128, 128], [1, 10]]),
            axis=AXIS.X,
        )
    for bt in range(n_bt):
        nc.vector.tensor_scalar(
            AP(hsall.tensor, bt * 16, [[128, 128], [1, 10]]),
            AP(lgall.tensor, bt * 16, [[128, 128], [1, 10]]),
            mxall[:, bt : bt + 1], None, ALU.subtract,
        )
    for bt in range(n_bt):
        nc.scalar.activation(
            AP(exall.tensor, bt * 16, [[128, 128], [1, 10]]),
            AP(hsall.tensor, bt * 16, [[128, 128], [1, 10]]),
            ACTF.Exp,
        )
    for bt in range(n_bt):
        nc.vector.reduce_sum(
            smal[:, bt : bt + 1],
            AP(exall.tensor, bt * 16, [[128, 128], [1, 10]]),
            axis=AXIS.X,
        )
    nc.scalar.activation(lnal[:, 0:n_bt], smal[:, 0:n_bt], ACTF.Ln)
    outt = P["smfpool"].tile([128, 128], F32, tag="outt")
    for bt in range(n_bt):
        nc.vector.tensor_scalar(
            AP(outt.tensor, bt * 16, [[128, 128], [1, 10]]),
            AP(hsall.tensor, bt * 16, [[128, 128], [1, 10]]),
            lnal[:, bt : bt + 1], None, ALU.subtract,
        )
    nc.sync.dma_start(
        out=AP(out_d, 0, [[10, 128], [1280, n_bt], [1, 10]]),
        in_=AP(outt.tensor, 0, [[128, 128], [16, n_bt], [1, 10]]),
    )


_CACHE = {}


def _get_nc(b_core):
    if b_core not in _CACHE:
        nc = bacc.Bacc("TRN2", target_bir_lowering=False, debug=False, num_devices=N_CORES)
        _build(nc, b_core)
        nc.compile()
        _CACHE[b_core] = nc
    return _CACHE[b_core]


def _prep_inputs(inputs):
    import ml_dtypes

    bf16 = ml_dtypes.bfloat16
    w1 = np.asarray(inputs["w1"], dtype=np.float32)  # [32, 1, 5, 5]
    w2 = np.asarray(inputs["w2"], dtype=np.float32)  # [64, 32, 5, 5]
    w3 = np.asarray(inputs["w3"], dtype=np.float32)  # [10, 64, 3, 3]
    b1 = np.asarray(inputs["b1"], dtype=np.float32)
    b2 = np.asarray(inputs["b2"], dtype=np.float32)
    b3 = np.asarray(inputs["b3"], dtype=np.float32)

    # conv1 lhsT: [ (dy8, dx5) dy-MAJOR (row = dy*5+dx), (rm4, co32) ]
    w1e = np.zeros((40, 128), dtype=np.float32)
    for dy in range(8):
        for dx in range(5):
            for rm in range(4):
                k = dy - rm
                if 0 <= k <= 4:
                    blk = (0, 2, 1, 3)[rm]  # row-pair interleave for pooling
                    w1e[dy * 5 + dx, blk * 32 : blk * 32 + 32] = w1[:, 0, k, dx]
    w1e = w1e.astype(bf16)

    # conv2 lhsT: [ (dy-parity, dx-parity, ci32), q*128 + (rm2, co64) ]
    # for the 9 base positions q = (dy2/2)*3 + dx2/2, dy2,dx2 in {0,2,4}
    w2qp = np.zeros((128, 1152), dtype=np.float32)
    for dy2 in (0, 2, 4):
        for dx2 in (0, 2, 4):
            q = (dy2 // 2) * 3 + dx2 // 2
            for gy in (0, 1):
                for gx in (0, 1):
                    g = gy * 2 + gx
                    dy, dx = dy2 + gy, dx2 + gx
                    if dx > 4:
                        continue
                    for rm in range(2):
                        k = dy - rm
                        if 0 <= k <= 4:
                            w2qp[
                                g * 32 : g * 32 + 32,
                                q * 128 + rm * 64 : q * 128 + rm * 64 + 64,
                            ] = w2[:, :, k, dx].T
    w2qp = w2qp.astype(bf16)

    # conv3 lhsT: [ (rep2, ci64), ti2*128 + (op4*32 + co10) ], GAP 1/4 folded
    w3e = np.zeros((128, 1024), dtype=np.float32)
    for ti2 in range(8):
        for rep in range(2):
            ip = 2 * ti2 + rep
            iy, ix = ip // 4, ip % 4
            for op in range(4):
                oy, ox = op // 2, op % 2
                ky, kx = iy - oy, ix - ox
                if 0 <= ky <= 2 and 0 <= kx <= 2:
                    w3e[
                        rep * 64 : rep * 64 + 64,
                        ti2 * 128 + op * 32 : ti2 * 128 + op * 32 + 10,
                    ] = 0.25 * w3[:, :, ky, kx].T
    w3e = w3e.astype(bf16)

    b1p = np.tile(b1, 4).reshape(128, 1)
    b2p = b2.reshape(64, 1)
    b3q = np.pad(0.25 * b3, (0, 6)).reshape(16, 1)
    ident10p = np.eye(16, dtype=np.float32)
    return dict(
        ident10p=ident10p,
        w1e=w1e,
        w2qp=w2qp,
        w3e=w3e,
        b1p=b1p,
        b2p=b2p,
        b3q=b3q,
    )


def _run(inputs, b_core=B_CORE, trace=False):
    import ml_dtypes

    fp8 = ml_dtypes.float8_e4m3
    x = np.ascontiguousarray(np.asarray(inputs["x"], dtype=np.float32))
    B = b_core * N_CORES
    ch = x[:B, 2]  # [B, 28, 28]
    img8 = np.zeros((B, 38, 38), dtype=fp8)
    img8[:, 5:33, 5:33] = ch.astype(fp8)
    img8 = img8.reshape(B, 1444)
    # dgall[s, t*128 + m] = delta(s mod 128, m) * fp8(tmpl[s, t])
    n_bt_total = B // 128
    tmpl8 = np.ascontiguousarray(ch[:, 8:19, 8:19]).reshape(n_bt_total, 128, 121).astype(fp8)
    dg = np.zeros((n_bt_total, 128, 121, 128), dtype=fp8)
    for p in range(128):
        dg[:, p, :, p] = tmpl8[:, p, :]
    dg = dg.reshape(B, 15488)

    consts = _prep_inputs(inputs)
    nc = _get_nc(b_core)
    in_maps = [
        {
            "img8p": img8[i * b_core : (i + 1) * b_core],
            "dgallp": dg[i * b_core : (i + 1) * b_core],
            **consts,
        }
        for i in range(N_CORES)
    ]
    res = run_bass_kernel_spmd(nc, in_maps, core_ids=list(range(N_CORES)), trace=trace)
    out = np.concatenate([res.results[i]["out"] for i in range(N_CORES)], axis=0)
    return out.astype(np.float32), res


def kernel(**inputs) -> np.ndarray:
    out, _ = _run(inputs)
    return out


# revision 56
# speedup vs baseline: 1.0675x; 1.0675x over previous
"""Trainium2 Bass kernel for nn_Net_76330158785143 (dense_cnn).

Pipeline per sample: per-sample 11x11 autocorrelation of channel 2 ->
conv5x5(1->32) relu -> maxpool2 -> conv5x5(32->64) relu -> maxpool2 ->
conv3x3(64->10) relu -> GAP -> log_softmax.

Sharding: pure data parallel, batch 8192 -> 1024 per core across 8 cores.

PE work is minimized by K/M-packing (matmul cost ~ out free size only):
 - corr: fp8(e4m3) DoubleRow diag matmuls, two taps per pass (pairs along
   x, plus pairs along y for the leftover column), each clipped to the
   union sub-rectangle its zero-padded shifts can reach. The diag
   matrices and the padded fp8 image are precomputed on the host and
   streamed in. Drained to an fp8 corr map on ACT (the next tile's first
   corr matmul waits on this; the DVE queue would bury it behind pools).
 - conv1: im2col via 40 shift-replicated tap partitions staged in fp8 at
   partitions 64..103 dy-MAJOR (partition = 64 + 5*dy + dx): the 8
   per-dy SBUF->SBUF gathers land on 8 distinct DMA ports, and the 4 dx
   shifts are partition-stride-5 contiguous copies. Staged 2 subs ahead
   (3 bufs); gathers split sync/gpsimd rings, shifts+replicas on the
   gpsimd (SWDGE) ring so the ACT/sync HWDGE rings stay short. K=40,
   M=128=(4 out rows x 32 ch), bf16 weights x fp8 moving. The matmul
   PSUM out-AP de-interleaves x-parity (free for the PE), so the
   relu+bias drains on ACT are fully contiguous and the maxpool runs as
   packed-2x DVE ops at equal partition bases.
 - conv2: 4 quarter-shifted input replicas {0,1 col; 0,1 row} give
   K=128=(2dy x 2dx x 32 ci), M=128=(2 out rows x 64 ch): 9 accumulating
   matmuls of N=512 per cc, bf16. PSUM out-AP de-interleaves x-parity;
   relu+bias on ACT straight from PSUM is contiguous; pool y then pool x
   run on DVE, the x-fold written twice straight into the l3 staging
   layout (rep1 shifted 1 pixel) so no l3 DMAs are needed.
 - conv3: K=128=(2 input-pixel reps, 64 ci), M=128=(4 out pix x co),
   8 matmuls of N=32; GAP 1/4 scale folded into the weights; drains on
   DVE to keep the ACT queue short for the PSUM-gating drains.
Scheduling: a flat rolling pipeline emits the corr matmuls of tile k
interleaved into the conv phases of tile k-1 (B-phases lag A-phases by
two sub-slots, crossing tile boundaries), so the PE fills every
ACT/pool/DMA-gated gap; img8/dgall prefetch one tile ahead of use.
"""

import sys

sys.path.insert(0, "/opt/trn_rl_repo")

import numpy as np

import concourse.bacc as bacc
import concourse.mybir as mybir
from concourse.ap import AP
from concourse.tile import TileContext
from concourse.bass_utils import run_bass_kernel_spmd

F32 = mybir.dt.float32
BF16 = mybir.dt.bfloat16
FP8 = mybir.dt.float8e4
DR = mybir.MatmulPerfMode.DoubleRow
ALU = mybir.AluOpType
ACTF = mybir.ActivationFunctionType
AXIS = mybir.AxisListType

N_CORES = 8
B_FULL = 8192
B_CORE = B_FULL // N_CORES

S40P = 18816  # s40 per-partition pitch: 32 samples x 588-elem windows
S40B = 64  # s40/w1e base partition (odd DMA ports, frees even for the rest)

# corr work units: (u, v, paired). Pairs (v, v+1) share one DoubleRow
# matmul; the unit containing the center tap (u=5, v=4/5 -> full union
# rect) is emitted first so start=True zeroes the whole PSUM bank.
_UNITS = []
for _u in range(11):
    for _v in (0, 2, 4, 6, 8):
        _UNITS.append((_u, _v, "v"))
for _u in (0, 2, 4, 6, 8):
    _UNITS.append((_u, 10, "u"))
_UNITS.append((10, 10, None))
_UNITS.remove((5, 4, "v"))
_UNITS.insert(0, (5, 4, "v"))


def _build(nc, b_core):
    """Emit the full per-core program for b_core samples (multiple of 128)."""
    n_bt = b_core // 128  # 128-sample tiles

    img8_d = nc.dram_tensor("img8p", [b_core, 1444], FP8, kind="ExternalInput")
    dgall_d = nc.dram_tensor("dgallp", [b_core, 15488], FP8, kind="ExternalInput")
    ident10p_d = nc.dram_tensor("ident10p", [16, 16], F32, kind="ExternalInput")
    w1e_d = nc.dram_tensor("w1e", [40, 128], BF16, kind="ExternalInput")
    w2qp_d = nc.dram_tensor("w2qp", [128, 1152], BF16, kind="ExternalInput")
    w3e_d = nc.dram_tensor("w3e", [128, 1024], BF16, kind="ExternalInput")
    b1p_d = nc.dram_tensor("b1p", [128, 1], F32, kind="ExternalInput")
    b2p_d = nc.dram_tensor("b2p", [64, 1], F32, kind="ExternalInput")
    b3q_d = nc.dram_tensor("b3q", [16, 1], F32, kind="ExternalInput")
    out_d = nc.dram_tensor("out", [b_core, 10], F32, kind="ExternalOutput")

    with TileContext(nc) as tc:
        cpool_cm = tc.tile_pool(name="const", bufs=1)
        cpool = cpool_cm.__enter__()

        def _load_const(name, dram, shape, dtype):
            t = cpool.tile(shape, dtype, name=name + "_sb")
            f = int(np.prod(shape[1:]))
            nc.sync.dma_start(
                out=AP(t.tensor, 0, [[f, shape[0]], [1, f]]),
                in_=AP(dram, 0, [[f, shape[0]], [1, f]]),
            )
            return t

        def _load_consts():
            # w1e lives at partitions S40B..S40B+39 to match the s40 K-window
            w1e_t = cpool.tile([128, 128], BF16, name="w1e_sb")
            nc.sync.dma_start(
                out=AP(w1e_t.tensor, S40B * 128, [[128, 40], [1, 128]]),
                in_=AP(w1e_d, 0, [[128, 40], [1, 128]]),
            )
            return dict(
                ident10=_load_const("ident10", ident10p_d, [16, 16], F32),
                w1e_sb=w1e_t,
                w2q_sb=_load_const("w2qp", w2qp_d, [128, 1152], BF16),
                w3e_sb=_load_const("w3e", w3e_d, [128, 1024], BF16),
                b1_sb=_load_const("b1p", b1p_d, [128, 1], F32),
                b2_sb=_load_const("b2p", b2p_d, [64, 1], F32),
                b3q_sb=_load_const("b3q", b3q_d, [16, 1], F32),
            )

        C = {}

        from contextlib import ExitStack

        with ExitStack() as stack:
            def _pool(name, bufs, space=None):
                kw = {"space": space} if space else {}
                return stack.enter_context(tc.tile_pool(name=name, bufs=bufs, **kw))

            P = dict(
                img8pool=_pool("img8", 2),
                dgallpool=_pool("dgall", 2),
                corrpool=_pool("corr", 2),
                s40pool=_pool("s40", 3),
                a1spool=_pool("a1s", 1),
                a1s2pool=_pool("a1s2", 1),
                pxspool=_pool("pxs", 1),
                dupApool=_pool("dupA", 3),
                m1pool=_pool("m1", 2),
                l3pool=_pool("l3", 2),
                smpool=_pool("sm", 2),
                lgbpool=_pool("lgb", 2),
                smfpool=_pool("smf", 1),
                pcorr=_pool("pcorr", 1, "PSUM"),
                pc1=_pool("pc1", 3, "PSUM"),
                pc2=_pool("pc2", 2, "PSUM"),
                pc3=_pool("pc3", 1, "PSUM"),
            )

            # Flat rolling pipeline. Per steady iteration k:
            #   P1: A0(k-1) B2(k-2) A1(k-1) B3(k-2) A2(k-1) B0(k-1),
            #       corr units of bt k filling the conv pipeline gaps
            #   P2: drain corr units, finish(k) -> corr(k), create
            #       state(k) (which prefetches s40(k,0) ahead of the
            #       replica DMAs emitted later in A3)
            #   P3: setup(k+1) so its units can fill P4
            #   P4: A3(k-1) B1(k-1)
            states = {}

            def mkfill(ce):
                it = iter(range(61))

                def fill(n, _it=it, _ce=ce):
                    got = []
                    for _ in range(n):
                        i = next(_it, None)
                        if i is None:
                            break
                        got.append(i)
                    if got:
                        _ce.units(got[0], got[-1] + 1)

                return fill

            def run(st, kind, s, fill):
                if st is None:
                    return
                if kind == "a":
                    st.phase_a(s, fill)
                else:
                    st.phase_b(s, fill)
                    if s == 3:
                        st.lgall = lgall
                        st.tail()
                        del states[st.bt]

            lgall = P["smfpool"].tile([128, 128], F32, tag="lgall")
            ce = _CorrEmitter(nc, 0, img8_d, dgall_d, P, C)
            ce.setup()
            # consts load after the first tile's img8/dgall so the first
            # corr matmuls aren't stalled behind them on the DMA queue
            C.update(_load_consts())
            fill = mkfill(ce)
            for k in range(n_bt):
                sa = states.get(k - 1)
                sb2 = states.get(k - 2)
                for st, kind, s in (
                    (sb2, "b", 2), (sa, "a", 0),
                    (sb2, "b", 3), (sa, "a", 1),
                    (sa, "b", 0), (sa, "a", 2),
                ):
                    run(st, kind, s, fill)
                # queue the next tile's img8/dgall loads ahead of the corr
                # drain block so they overlap ~30us of PE work before the
                # first unit of tile k+1 needs them
                ce_next = None
                if k + 1 < n_bt:
                    ce_next = _CorrEmitter(nc, k + 1, img8_d, dgall_d, P, C)
                    ce_next.setup()
                fill(61)
                corr = ce.finish()
                if ce_next is not None:
                    ce = ce_next
                    fill = mkfill(ce)
                else:
                    fill = None
                states[k] = _ConvState(nc, k, corr, out_d, P, C)
                run(sa, "b", 1, fill)
                run(sa, "a", 3, fill)
            # drain: conv phases of the last two tiles, no corr fill left
            for k in (n_bt, n_bt + 1):
                sa = states.get(k - 1)
                sb2 = states.get(k - 2)
                for st, kind, s in (
                    (sb2, "b", 2), (sa, "a", 0),
                    (sb2, "b", 3), (sa, "a", 1),
                    (sa, "b", 0), (sa, "a", 2),
                    (sa, "b", 1), (sa, "a", 3),
                ):
                    run(st, kind, s, None)
            _softmax_final(nc, n_bt, lgall, out_d, P)
        cpool_cm.__exit__(None, None, None)
    return nc


class _CorrEmitter:
    """Per-sample 11x11 correlation of 128 samples via fp8 DoubleRow."""

    def __init__(self, nc, bt, img8_d, dgall_d, P, C):
        self.nc = nc
        self.bt = bt
        self.img8_d = img8_d
        self.dgall_d = dgall_d
        self.P = P
        self.C = C

    def setup(self):
        nc, P = self.nc, self.P
        img8 = P["img8pool"].tile([128, 1444], FP8)
        nc.sync.dma_start(
            out=AP(img8.tensor, 0, [[1444, 128], [1, 1444]]),
            in_=AP(self.img8_d, self.bt * 128 * 1444, [[1444, 128], [1, 1444]]),
        )
        dgall = P["dgallpool"].tile([128, 15488], FP8)
        nc.sync.dma_start(
            out=AP(dgall.tensor, 0, [[15488, 128], [1, 15488]]),
            in_=AP(self.dgall_d, self.bt * 128 * 15488, [[15488, 128], [1, 15488]]),
        )
        self.dgall = dgall
        self.img8 = img8
        self.ps_a = P["pcorr"].tile([128, 392], F32, tag="corr_a")
        self.ps_b = P["pcorr"].tile([128, 392], F32, tag="corr_b")

    def units(self, lo, hi):
        nc = self.nc
        img8 = self.img8
        for i in range(lo, hi):
            u, v, paired = _UNITS[i]
            # valid output rows r: img_pad rows r+u in [5, 32]; "v" pairs
            # (v, v+1) in one DoubleRow pass, "u" pairs (u, u+1)
            if paired == "u":
                r0 = min(max(0, 5 - u), max(0, 5 - u - 1))
                r1 = max(min(27, 32 - u), min(27, 32 - u - 1))
            else:
                r0, r1 = max(0, 5 - u), min(27, 32 - u)
            if paired == "v":
                c0 = min(max(0, 5 - v), max(0, 5 - v - 1))
                c1 = max(min(27, 32 - v), min(27, 32 - v - 1))
            else:
                c0, c1 = max(0, 5 - v), min(27, 32 - v)
            ncol = c1 - c0 + 1
            t = u * 11 + v
            if paired == "v":
                lhs = AP(self.dgall.tensor, t * 128, [[15488, 128], [128, 2], [1, 128]])
            elif paired == "u":
                lhs = AP(self.dgall.tensor, t * 128, [[15488, 128], [11 * 128, 2], [1, 128]])
            else:
                lhs = AP(self.dgall.tensor, t * 128, [[15488, 128], [1, 128]])
            kst = {"v": 1, "u": 38}.get(paired)
            for bank, (b0, b1, ps) in enumerate(
                ((r0, min(r1, 13), self.ps_a), (max(r0, 14), r1, self.ps_b))
            ):
                if b0 > b1:
                    continue
                nr = b1 - b0 + 1
                out = AP(
                    ps.tensor,
                    (b0 - 14 * bank) * 28 + c0,
                    [[392, 128], [28, nr], [1, ncol]],
                )
                if paired:
                    rhs = AP(
                        img8.tensor,
                        (u + b0) * 38 + v + c0,
                        [[1444, 128], [kst, 2], [38, nr], [1, ncol]],
                    )
                    nc.tensor.matmul(
                        out, lhs, rhs,
                        start=(i == 0), stop=(i == 60),
                        perf_mode=DR, skip_group_check=True,
                    )
                else:
                    rhs = AP(
                        img8.tensor,
                        (u + b0) * 38 + v + c0,
                        [[1444, 128], [38, nr], [1, ncol]],
                    )
                    nc.tensor.matmul(
                        out, lhs, rhs,
                        start=False, stop=(i == 60),
                        skip_group_check=True,
                    )

    def finish(self):
        nc, P = self.nc, self.P
        # fp8 corr map: feeds conv1's moving operand (weights stay bf16);
        # halves the im2col staging bytes. Drain on ACT: the next tile's
        # first corr matmul waits on these (pcorr WAR), and the DVE queue
        # buries them behind pool bursts while ACT drains promptly.
        corr = P["corrpool"].tile([128, 784], FP8)
        nc.scalar.activation(corr[:, 0:392], self.ps_a[:, :], ACTF.Copy)
        nc.scalar.activation(corr[:, 392:784], self.ps_b[:, :], ACTF.Copy)
        return corr


class _ConvState:
    """conv1 -> pool -> conv2 -> pool -> conv3 -> GAP -> log_softmax for one
    128-sample tile, split into per-32-sample A (im2col+conv1+pool+replicas)
    and B (conv2+pool+conv3+GAP) phases."""

    def __init__(self, nc, bt, corr, out_d, P, C):
        self.nc = nc
        self.bt = bt
        self.corr = corr
        self.out_d = out_d
        self.P = P
        self.C = C
        self.logitsb = P["lgbpool"].tile([16, 128], F32)
        self.dups = [None] * 4
        self.s40s = [None] * 4
        self._s40(0)
        self._s40(1)

    def _s40(self, sub):
        """Stage the conv1 im2col window for one 32-sample sub at partitions
        S40B..S40B+39, dy-MAJOR (partition = S40B + 5*dy + dx): one dy-
        gather from the DRAM corr copy (dest partitions stride 5 land on 8
        distinct DMA ports), then 4 partition-stride-5 contiguous dx-shift
        copies (tail elements that wrap into the next sample's window are
        never read). Shifts alternate the two HWDGE rings (sync/scalar)."""
        nc, P = self.nc, self.P
        s40 = P["s40pool"].tile([128, S40P], FP8)
        for dy in range(8):
            eng = nc.sync if dy % 2 == 0 else nc.gpsimd
            eng.dma_start(
                out=AP(s40.tensor, (S40B + 5 * dy) * S40P, [[S40P, 1], [1, S40P]]),
                in_=AP(
                    self.corr.tensor,
                    sub * 32 * 784 + dy * 28,
                    [[784, 32], [1, 588]],
                ),
            )
        for dx in range(1, 5):
            nc.gpsimd.dma_start(
                out=AP(s40.tensor, (S40B + dx) * S40P, [[5 * S40P, 8], [1, S40P - 4]]),
                in_=AP(s40.tensor, S40B * S40P + dx, [[5 * S40P, 8], [1, S40P - 4]]),
            )
        self.s40s[sub] = s40

    def phase_a(self, sub, fill=None):
        nc, P, C = self.nc, self.P, self.C
        if sub < 2:
            self._s40(sub + 2)  # prefetch 2 subs ahead (s40pool bufs=3)
        s40 = self.s40s[sub]
        # conv1: K=40 (partitions S40B..), M=(rm4, co32); the PSUM out-AP
        # de-interleaves x-parity so drains are contiguous and land in the
        # two half tiles (samples 0-17 / 18-31) at equal partition bases.
        dupA = P["dupApool"].tile([128, 4624], BF16)
        nc.gpsimd.memset(dupA[0:32, 4608:4624], 0.0)
        a1A = P["a1spool"].tile([128, 2592], BF16)
        a1B = P["a1s2pool"].tile([128, 2016], BF16)
        for g in range(11):
            s0 = 3 * g
            ns = 3 if g < 10 else 2
            nf = ns * 144
            ps1 = P["pc1"].tile([128, 432], F32, tag="ps1")
            nc.tensor.matmul(
                AP(ps1.tensor, 0, [[432, 128], [12, ns * 6], [1, 12], [ns * 72, 2]]),
                C["w1e_sb"][S40B : S40B + 40, :],
                AP(
                    s40.tensor,
                    S40B * S40P + s0 * 588,
                    [[S40P, 40], [588, ns], [112, 6], [1, 24]],
                ),
                start=True,
                stop=True,
            )
            # relu+bias, contiguous PSUM read / 216-elem runs out. All on
            # ACT: the PSUM-bank WAR for matmul g+3 waits on drain g, and
            # ACT completes promptly while DVE buries drains behind pools
            # (splitting any of these onto DVE measured ~7% slower overall).
            half, hoff, hp = (a1A, g * 216, 2592) if g < 6 else (a1B, (g - 6) * 216, 2016)
            nc.scalar.activation(
                AP(half.tensor, hoff, [[hp, 128], [hp // 2, 2], [1, ns * 72]]),
                AP(ps1.tensor, 0, [[432, 128], [1, nf]]),
                ACTF.Relu, bias=C["b1_sb"][:, 0:1],
            )
            if g == 5:
                self._pools(0, 108, a1A, 2592, dupA)
            elif g == 10:
                self._pools(108, 84, a1B, 2016, dupA)
                # replicas for 4-tap K-packing: group g=(gy,gx) holds the
                # pooled map shifted by 12*gy + gx elems
                for g2, sh in ((1, 1), (2, 12), (3, 13)):
                    nc.gpsimd.dma_start(
                        out=AP(dupA.tensor, 32 * g2 * 4624, [[4624, 32], [1, 4596]]),
                        in_=AP(dupA.tensor, sh, [[4624, 32], [1, 4596]]),
                    )
            if fill is not None and (g % 2 == 1 or g == 10):
                fill(3 if g == 10 else 1)
        self.dups[sub] = dupA

    def _pools(self, sq0, nsq, a1h, hp, dupA):
        """maxpool 2x2 over nsq (sample,quad) units starting at fused unit
        sq0. conv1's M layout interleaves row-pairs [rm0, rm2, rm1, rm3],
        so x-pooling runs as two 64-wide ops (per-partition cost only) and
        the y-folds pair the two x-halves of one tile at equal bases
        (parts 0:32 = rm0 vs rm1 -> even rows, 32:64 = rm2 vs rm3 -> odd)."""
        nc, P = self.nc, self.P
        pxall = P["pxspool"].tile([64, 4608], BF16, tag="pxall")
        for grp in range(2):
            nc.vector.tensor_max(
                AP(pxall.tensor, grp * 2304 + sq0 * 12, [[4608, 64], [1, nsq * 12]]),
                AP(a1h.tensor, grp * 64 * hp, [[hp, 64], [1, nsq * 12]]),
                AP(a1h.tensor, grp * 64 * hp + hp // 2, [[hp, 64], [1, nsq * 12]]),
            )
        nc.vector.tensor_max(
            AP(dupA.tensor, sq0 * 24, [[4624, 32], [24, nsq], [1, 12]]),
            AP(pxall.tensor, sq0 * 12, [[4608, 32], [12, nsq], [1, 12]]),
            AP(pxall.tensor, 2304 + sq0 * 12, [[4608, 32], [12, nsq], [1, 12]]),
        )
        nc.vector.tensor_max(
            AP(dupA.tensor, sq0 * 24 + 12, [[4624, 32], [24, nsq], [1, 12]]),
            AP(pxall.tensor, 32 * 4608 + sq0 * 12, [[4608, 32], [12, nsq], [1, 12]]),
            AP(pxall.tensor, 32 * 4608 + 2304 + sq0 * 12, [[4608, 32], [12, nsq], [1, 12]]),
        )

    def phase_b(self, sub, fill=None):
        nc, P, C = self.nc, self.P, self.C
        dupA = self.dups[sub]
        l3 = None
        for cc in range(2):
            ps2 = P["pc2"].tile([128, 512], F32, tag="ps2")
            for q in range(9):
                dy2, dx2 = (q // 3) * 2, (q % 3) * 2
                nc.tensor.matmul(
                    AP(ps2.tensor, 0, [[512, 128], [4, 64], [1, 4], [256, 2]]),
                    C["w2q_sb"][:, q * 128 : q * 128 + 128],
                    AP(dupA.tensor, cc * 2304 + dy2 * 12 + dx2,
                       [[4624, 128], [144, 16], [24, 4], [1, 8]]),
                    start=(q == 0),
                    stop=(q == 8),
                )
                # a corr unit between accumulating matmuls targets other
                # PSUM banks, turning the same-bank drain-wait into work
                if fill is not None and q in (2, 5, 7):
                    fill(1)
            # relu+bias straight from PSUM on ACT (relu commutes with the
            # maxpool); the matmul out-AP already de-interleaved x-parity
            # so both the drain and the pool stages below are contiguous
            rr2 = [
                P["m1pool"].tile([64, 512], BF16, tag=f"rr2{h}", name=f"rr2{h}")
                for h in range(2)
            ]
            for h in range(2):
                nc.scalar.activation(
                    rr2[h][:, :],
                    AP(ps2.tensor, h * 64 * 512, [[512, 64], [1, 512]]),
                    ACTF.Relu, bias=C["b2_sb"][:, 0:1],
                )
            # pool y: rm fold; pool x: parity-half fold written TWICE by
            # DVE straight into the l3 staging layout [128=(2 reps, 64ci),
            # (32s,16pix)] -- rep1 shifted by 1 pixel (no DMA needed; only
            # even l3 columns are ever read by the conv3 matmuls)
            m1 = P["m1pool"].tile([64, 512], BF16, tag="m1")
            nc.vector.tensor_max(m1[:, :], rr2[0][:, :], rr2[1][:, :])
            if cc == 0:
                l3 = P["l3pool"].tile([128, 512], BF16, name="l3")
            nc.vector.tensor_max(
                AP(l3.tensor, cc * 256, [[512, 64], [1, 256]]),
                m1[:, 0:256], m1[:, 256:512],
            )
            nc.vector.tensor_max(
                AP(l3.tensor, 64 * 512 + cc * 256, [[512, 64], [1, 255]]),
                m1[:, 1:256], m1[:, 257:512],
            )
            if fill is not None:
                fill(1)
        # conv3: K=128=(rep2, ci64), M=128=(op4 x 32), 8 matmuls of N=32
        ps3 = P["pc3"].tile([128, 48], F32, tag="ps3", name="ps3")
        for ti2 in range(8):
            nc.tensor.matmul(
                ps3[:, 0:32],
                C["w3e_sb"][:, ti2 * 128 : ti2 * 128 + 128],
                AP(l3.tensor, 2 * ti2, [[512, 128], [16, 32]]),
                start=(ti2 == 0),
                stop=(ti2 == 7),
            )
        # relu(ps3 + b3/4) per output pixel, then GAP = sum of the 4 pixels
        rr = [
            P["smpool"].tile([16, 32], F32, tag=f"rr{op}", name=f"rr{op}")
            for op in range(4)
        ]
        for op in range(4):
            nc.vector.tensor_scalar(
                rr[op][0:10, :], ps3[op * 32 : op * 32 + 10, 0:32],
                C["b3q_sb"][0:10, 0:1], 0.0, ALU.add, ALU.max,
            )
        t1 = P["smpool"].tile([16, 32], F32, tag="t1")
        nc.vector.tensor_add(t1[0:10, :], rr[0][0:10, :], rr[1][0:10, :])
        t2 = P["smpool"].tile([16, 32], F32, tag="t2")
        nc.vector.tensor_add(t2[0:10, :], rr[2][0:10, :], rr[3][0:10, :])
        nc.vector.tensor_add(
            self.logitsb[0:10, sub * 32 : sub * 32 + 32], t1[0:10, :], t2[0:10, :]
        )

    def tail(self):
        nc, P, C = self.nc, self.P, self.C
        psT = P["pc3"].tile([128, 48], F32, tag="ps3", name="psT")
        nc.tensor.transpose(psT[:, 0:10], self.logitsb[0:10, :], C["ident10"][0:10, 0:10])
        nc.vector.tensor_copy(
            out=AP(self.lgall.tensor, self.bt * 16, [[128, 128], [1, 10]]),
            in_=psT[:, 0:10],
        )


def _softmax_final(nc, n_bt, lgall, out_d, P):
    """Batched log_softmax for all tiles: groups each activation function
    into one run so the ACT table loads twice total instead of ~3x per
    tile mid-stream."""
    mxall = P["smfpool"].tile([128, 8], F32, tag="mxall")
    hsall = P["smfpool"].tile([128, 128], F32, tag="hsall")
    exall = P["smfpool"].tile([128, 128], F32, tag="exall")
    smal = P["smfpool"].tile([128, 8], F32, tag="smal")
    lnal = P["smfpool"].tile([128, 8], F32, tag="lnal")
    for bt in range(n_bt):
        nc.vector.reduce_max(
            mxall[:, bt : bt + 1],
            AP(lgall.tensor, bt * 16, [[128, 128], [1, 10]]),
            axis=AXIS.X,
        )
    for bt in range(n_bt):
        nc.vector.tensor_scalar(
            AP(hsall.tensor, bt * 16, [[128, 128], [1, 10]]),
            AP(lgall.tensor, bt * 16, [[128, 128], [1, 10]]),
            mxall[:, bt : bt + 1], None, ALU.subtract,
        )
    for bt in range(n_bt):
        nc.scalar.activation(
            AP(exall.tensor, bt * 16, [[128, 128], [1, 10]]),
            AP(hsall.tensor, bt * 16, [[128, 128], [1, 10]]),
            ACTF.Exp,
        )
    for bt in range(n_bt):
        nc.vector.reduce_sum(
            smal[:, bt : bt + 1],
            AP(exall.tensor, bt * 16, [[128, 128], [1, 10]]),
            axis=AXIS.X,
        )
    nc.scalar.activation(lnal[:, 0:n_bt], smal[:, 0:n_bt], ACTF.Ln)
    outt = P["smfpool"].tile([128, 128], F32, tag="outt")
    for bt in range(n_bt):
        nc.vector.tensor_scalar(
            AP(outt.tensor, bt * 16, [[128, 128], [1, 10]]),
            AP(hsall.tensor, bt * 16, [[128, 128], [1, 10]]),
            lnal[:, bt : bt + 1], None, ALU.subtract,
        )
    nc.sync.dma_start(
        out=AP(out_d, 0, [[10, 128], [1280, n_bt], [1, 10]]),
        in_=AP(outt.tensor, 0, [[128, 128], [16, n_bt], [1, 10]]),
    )


_CACHE = {}


def _get_nc(b_core):
    if b_core not in _CACHE:
        nc = bacc.Bacc("TRN2", target_bir_lowering=False, debug=False, num_devices=N_CORES)
        _build(nc, b_core)
        nc.compile()
        _CACHE[b_core] = nc
    return _CACHE[b_core]


def _prep_inputs(inputs):
    import ml_dtypes

    bf16 = ml_dtypes.bfloat16
    w1 = np.asarray(inputs["w1"], dtype=np.float32)  # [32, 1, 5, 5]
    w2 = np.asarray(inputs["w2"], dtype=np.float32)  # [64, 32, 5, 5]
    w3 = np.asarray(inputs["w3"], dtype=np.float32)  # [10, 64, 3, 3]
    b1 = np.asarray(inputs["b1"], dtype=np.float32)
    b2 = np.asarray(inputs["b2"], dtype=np.float32)
    b3 = np.asarray(inputs["b3"], dtype=np.float32)

    # conv1 lhsT: [ (dy8, dx5) dy-MAJOR (row = dy*5+dx), (rm4, co32) ]
    w1e = np.zeros((40, 128), dtype=np.float32)
    for dy in range(8):
        for dx in range(5):
            for rm in range(4):
                k = dy - rm
                if 0 <= k <= 4:
                    blk = (0, 2, 1, 3)[rm]  # row-pair interleave for pooling
                    w1e[dy * 5 + dx, blk * 32 : blk * 32 + 32] = w1[:, 0, k, dx]
    w1e = w1e.astype(bf16)

    # conv2 lhsT: [ (dy-parity, dx-parity, ci32), q*128 + (rm2, co64) ]
    # for the 9 base positions q = (dy2/2)*3 + dx2/2, dy2,dx2 in {0,2,4}
    w2qp = np.zeros((128, 1152), dtype=np.float32)
    for dy2 in (0, 2, 4):
        for dx2 in (0, 2, 4):
            q = (dy2 // 2) * 3 + dx2 // 2
            for gy in (0, 1):
                for gx in (0, 1):
                    g = gy * 2 + gx
                    dy, dx = dy2 + gy, dx2 + gx
                    if dx > 4:
                        continue
                    for rm in range(2):
                        k = dy - rm
                        if 0 <= k <= 4:
                            w2qp[
                                g * 32 : g * 32 + 32,
                                q * 128 + rm * 64 : q * 128 + rm * 64 + 64,
                            ] = w2[:, :, k, dx].T
    w2qp = w2qp.astype(bf16)

    # conv3 lhsT: [ (rep2, ci64), ti2*128 + (op4*32 + co10) ], GAP 1/4 folded
    w3e = np.zeros((128, 1024), dtype=np.float32)
    for ti2 in range(8):
        for rep in range(2):
            ip = 2 * ti2 + rep
            iy, ix = ip // 4, ip % 4
            for op in range(4):
                oy, ox = op // 2, op % 2
                ky, kx = iy - oy, ix - ox
                if 0 <= ky <= 2 and 0 <= kx <= 2:
                    w3e[
                        rep * 64 : rep * 64 + 64,
                        ti2 * 128 + op * 32 : ti2 * 128 + op * 32 + 10,
                    ] = 0.25 * w3[:, :, ky, kx].T
    w3e = w3e.astype(bf16)

    b1p = np.tile(b1, 4).reshape(128, 1)
    b2p = b2.reshape(64, 1)
    b3q = np.pad(0.25 * b3, (0, 6)).reshape(16, 1)
    ident10p = np.eye(16, dtype=np.float32)
    return dict(
        ident10p=ident10p,
        w1e=w1e,
        w2qp=w2qp,
        w3e=w3e,
        b1p=b1p,
        b2p=b2p,
        b3q=b3q,
    )


def _run(inputs, b_core=B_CORE, trace=False):
    import ml_dtypes

    fp8 = ml_dtypes.float8_e4m3
    x = np.ascontiguousarray(np.asarray(inputs["x"], dtype=np.float32))
    B = b_core * N_CORES
    ch = x[:B, 2]  # [B, 28, 28]
    img8 = np.zeros((B, 38, 38), dtype=fp8)
    img8[:, 5:33, 5:33] = ch.astype(fp8)
    img8 = img8.reshape(B, 1444)
    # dgall[s, t*128 + m] = delta(s mod 128, m) * fp8(tmpl[s, t])
    n_bt_total = B // 128
    tmpl8 = np.ascontiguousarray(ch[:, 8:19, 8:19]).reshape(n_bt_total, 128, 121).astype(fp8)
    dg = np.zeros((n_bt_total, 128, 121, 128), dtype=fp8)
    for p in range(128):
        dg[:, p, :, p] = tmpl8[:, p, :]
    dg = dg.reshape(B, 15488)

    consts = _prep_inputs(inputs)
    nc = _get_nc(b_core)
    in_maps = [
        {
            "img8p": img8[i * b_core : (i + 1) * b_core],
            "dgallp": dg[i * b_core : (i + 1) * b_core],
            **consts,
        }
        for i in range(N_CORES)
    ]
    res = run_bass_kernel_spmd(nc, in_maps, core_ids=list(range(N_CORES)), trace=trace)
    out = np.concatenate([res.results[i]["out"] for i in range(N_CORES)], axis=0)
    return out.astype(np.float32), res


def kernel(**inputs) -> np.ndarray:
    out, _ = _run(inputs)
    return out
